# revision 26
# baseline (speedup 1.0000x reference)
"""Luong 'concat' attention (nn_LuongAttention) — Trainium2 Bass kernel.

Strategy (8 NeuronCores, no collectives):
  - Shard (batch, tgt): core c handles batch c//2, tgt rows (c%2)*128..+128.
  - Sparse-attention mask compaction on host: only src positions with
    mask==1 (plus padding to a multiple of 16) are shipped/computed.
    Padding columns are zeroed via an explicit mask multiply before the
    softmax normalization, so results are exact for any mask.
  - Per core the device computes:
      e_projT[o,s] = We @ e_g.T           (PE, fp16 in / fp32 acc)
      hbT[o,t]    = Wh @ h.T + b          (PE + DVE)
      energy[o, s*T+t] = hbT[:,t] + e_projT[:,s]   (DVE tensor_scalar, fp16)
      tanh(energy)                         (ACT, big slabs, fp16 out)
      score[t,s] = sum_o v[o]*tanh(...)    (PE: fp16 weights, rhs=v, fp32 acc)
      masked softmax over s                (DVE/ACT, fp32)
      context = probs @ e_g                (PE, fp32)
  - Host scatters compacted probs back to [B,TGT,SRC] and stacks context.

fp16 is used only for the energy mid-section: fp32 matmuls on TRN2 run as
two LOW/HIGH passes with per-pass LDWEIGHTS, which makes the per-(s,o_tile)
energy-as-weights reduction ~6x more expensive than fp16 (which gets FWL
and a single pass). Softmax + context stay fp32.
"""

import numpy as np
from contextlib import ExitStack

import concourse.bass as bass
import concourse.tile as tile
from concourse import bacc, mybir
from concourse import bass_utils

N_CORES = 8
B, TGT, SRC, SS, TS = 4, 256, 256, 512, 512
T_C = TGT * B // N_CORES  # tgt rows per core = 128
O_TILES = TS // 128  # 4
D_TILES = SS // 128  # 4
SLAB_S = 32   # src positions per (slab, o_tile) DVE/ACT chunk
FUSED_S = 18  # s-columns whose add+tanh run fused on ACT (DVE/ACT balance)

FP = mybir.dt.float32
HP = mybir.dt.float16


def _chunks(total, size):
    out = []
    off = 0
    while off < total:
        out.append((off, min(size, total - off)))
        off += size
    return out


def _build(S_c):
    """Trace + compile the per-core program for compacted src size S_c."""
    nc = bacc.Bacc("TRN2", target_bir_lowering=False, debug=False,
                   num_devices=N_CORES)

    weT = nc.dram_tensor("weT", [SS, TS], HP, kind="ExternalInput").ap()
    whT = nc.dram_tensor("whT", [TS, TS], HP, kind="ExternalInput").ap()
    eTg = nc.dram_tensor("eTg", [SS, S_c], HP, kind="ExternalInput").ap()
    eg = nc.dram_tensor("eg", [S_c, SS], FP, kind="ExternalInput").ap()
    hT = nc.dram_tensor("hT", [TS, T_C], HP, kind="ExternalInput").ap()
    v4 = nc.dram_tensor("v4", [128, O_TILES], HP, kind="ExternalInput").ap()
    # smalls = [b4 | maskrep | identity] merged into one fp32 DMA
    smalls = nc.dram_tensor("smalls", [128, O_TILES + S_c + 128], FP,
                            kind="ExternalInput").ap()
    ctx_out = nc.dram_tensor("ctx_out", [T_C, SS], FP, kind="ExternalOutput").ap()
    probs_out = nc.dram_tensor("probs_out", [T_C, S_c], FP,
                               kind="ExternalOutput").ap()

    s_chunks = _chunks(S_c, 128)

    with tile.TileContext(nc) as tc, ExitStack() as ctx:
        consts = ctx.enter_context(tc.tile_pool(name="consts", bufs=1))
        slabs = ctx.enter_context(tc.tile_pool(name="slabs", bufs=4))
        p3 = ctx.enter_context(tc.tile_pool(name="p3", bufs=1))
        psum = ctx.enter_context(tc.tile_pool(name="psum", bufs=1, space="PSUM"))
        psum2 = ctx.enter_context(tc.tile_pool(name="psum2", bufs=2, space="PSUM"))

        # ---- Phase 0: loads ------------------------------------------------
        # Big loads on different engines' DMA queues so they run in parallel.
        weT_sb = consts.tile([128, D_TILES, TS], HP)
        nc.sync.dma_start(out=weT_sb, in_=weT.rearrange("(dt p) o -> p dt o", p=128))
        whT_sb = consts.tile([128, D_TILES, TS], HP)
        nc.gpsimd.dma_start(out=whT_sb, in_=whT.rearrange("(dt p) o -> p dt o", p=128))
        eTg_sb = consts.tile([128, D_TILES, S_c], HP)
        nc.scalar.dma_start(out=eTg_sb, in_=eTg.rearrange("(dt p) s -> p dt s", p=128))
        hT_sb = consts.tile([128, D_TILES, T_C], HP)
        nc.gpsimd.dma_start(out=hT_sb, in_=hT.rearrange("(dt p) t -> p dt t", p=128))
        v4_sb = consts.tile([128, O_TILES], HP)
        nc.sync.dma_start(out=v4_sb, in_=v4)
        smalls_sb = consts.tile([128, O_TILES + S_c + 128], FP)
        nc.gpsimd.dma_start(out=smalls_sb, in_=smalls)
        b4_sb = smalls_sb[:, 0:O_TILES]
        mask_sb = smalls_sb[:, O_TILES:O_TILES + S_c]
        ident = smalls_sb[:, O_TILES + S_c:]
        eg_sb = []
        for off, sz in s_chunks:
            t = consts.tile([sz, SS], FP, tag=f"eg{off}")
            nc.scalar.dma_start(out=t, in_=eg[off:off + sz, :])
            eg_sb.append(t)

        # ---- Phase 1: projections -----------------------------------------
        e_projT = []  # [128, S_c] fp16 per o_tile ; o = ot*128 + p
        hbT = []      # [128, T_C] fp16 per o_tile (h_proj + b)
        for ot in range(O_TILES):
            # h-projection first: its DVE b-add overlaps the e-proj matmuls,
            # so the first slab add can start sooner.
            ph = psum2.tile([128, T_C], FP, tag="ph_proj")
            for dt in range(D_TILES):
                nc.tensor.matmul(ph, whT_sb[:, dt, ot * 128:(ot + 1) * 128],
                                 hT_sb[:, dt, :],
                                 start=(dt == 0), stop=(dt == D_TILES - 1))
            ht = consts.tile([128, T_C], HP, tag=f"hbT{ot}")
            nc.vector.tensor_scalar_add(ht, ph, b4_sb[:, ot:ot + 1])
            hbT.append(ht)

            pe = psum2.tile([128, S_c], FP, tag="pe_proj")
            for dt in range(D_TILES):
                nc.tensor.matmul(pe, weT_sb[:, dt, ot * 128:(ot + 1) * 128],
                                 eTg_sb[:, dt, :],
                                 start=(dt == 0), stop=(dt == D_TILES - 1))
            # fp32: tensor_scalar requires a float32 scalar operand
            et = consts.tile([128, S_c], FP, tag=f"eprojT{ot}")
            nc.scalar.copy(out=et, in_=pe)
            e_projT.append(et)

        # ---- Phase 2: energy / tanh / score -------------------------------
        # s-columns [0, n_slab) take the DVE-add + big-slab-ACT-tanh path;
        # s-columns [n_slab, S_c) run add+tanh fused in one ACT op each
        # (bias = e_projT column), balancing DVE vs ACT occupancy.
        n_fused = min(FUSED_S, S_c // 2)
        n_slab = S_c - n_fused
        score = psum.tile([128, S_c], FP)  # [t, s]
        mm_count = [0]

        def score_mm(s, ot, lhsT):
            # One accumulation group for the whole phase: start=True clears
            # the entire PSUM bank, so only the very first matmul carries it.
            i = mm_count[0]
            nc.tensor.matmul(score[:, s:s + 1], lhsT, v4_sb[:, ot:ot + 1],
                             start=(i == 0), stop=(i == S_c * O_TILES - 1))
            mm_count[0] = i + 1

        def fused_unit(s):
            for ot in range(O_TILES):
                ef = slabs.tile([128, T_C], HP, tag="efused")
                nc.scalar.activation(out=ef, in_=hbT[ot],
                                     func=mybir.ActivationFunctionType.Tanh,
                                     bias=e_projT[ot][:, s:s + 1], scale=1.0)
                score_mm(s, ot, ef)

        # Interleave: fused s-columns are spread across slab iterations so
        # ACT's fused work overlaps DVE's slab adds instead of tailing.
        slab_starts = list(range(0, n_slab, SLAB_S))
        fused_cols = list(range(n_slab, S_c))
        for k, s0 in enumerate(slab_starts):
            ns = min(SLAB_S, n_slab - s0)
            for ot in range(O_TILES):
                ein = slabs.tile([128, SLAB_S * T_C], HP, tag="ein")
                for j in range(ns):
                    nc.vector.tensor_scalar_add(
                        ein[:, j * T_C:(j + 1) * T_C], hbT[ot],
                        e_projT[ot][:, s0 + j:s0 + j + 1])
                eout = slabs.tile([128, SLAB_S * T_C], HP, tag="eout")
                nc.scalar.activation(out=eout[:, :ns * T_C], in_=ein[:, :ns * T_C],
                                     func=mybir.ActivationFunctionType.Tanh)
                for j in range(ns):
                    score_mm(s0 + j, ot, eout[:, j * T_C:(j + 1) * T_C])
                # Front-load fused columns into the earlier slots so ACT's
                # independent work fills pipeline-fill gaps instead of
                # trailing after DVE finishes.
                idx = k * O_TILES + ot
                eff = max(1, len(slab_starts) * O_TILES - 4)
                if idx < eff:
                    lo = len(fused_cols) * idx // eff
                    hi = len(fused_cols) * (idx + 1) // eff
                    for s in fused_cols[lo:hi]:
                        fused_unit(s)

        # ---- Phase 3: masked softmax + context ----------------------------
        m = p3.tile([128, 1], FP)
        nc.vector.reduce_max(out=m, in_=score, axis=mybir.AxisListType.X)
        negm = p3.tile([128, 1], FP)
        nc.vector.tensor_scalar_mul(negm, m, -1.0)
        pm = p3.tile([128, S_c], FP)  # exp(score - max), then masked
        nc.scalar.activation(out=pm, in_=score,
                             func=mybir.ActivationFunctionType.Exp,
                             bias=negm, scale=1.0)
        nc.vector.tensor_mul(pm, pm, mask_sb)
        ssum = p3.tile([128, 1], FP)
        nc.vector.reduce_sum(out=ssum, in_=pm, axis=mybir.AxisListType.X)
        # Guard a fully-masked row (sum==0 -> inf -> NaN); reference yields 0.
        nc.vector.tensor_scalar_max(ssum, ssum, 1e-30)
        rinv = p3.tile([128, 1], FP)
        nc.vector.reciprocal(rinv, ssum)

        pr = p3.tile([128, S_c], FP)
        nc.vector.tensor_scalar_mul(pr, pm, rinv)
        nc.sync.dma_start(out=probs_out, in_=pr[:T_C, :])

        cps = psum.tile([128, SS], FP)
        for j, (off, sz) in enumerate(s_chunks):
            ptp = psum2.tile([sz, 128], FP, tag="ptp")
            nc.tensor.transpose(ptp, pm[:, off:off + sz], ident)
            pts = p3.tile([sz, 128], FP, tag=f"pts{j}")
            nc.vector.tensor_copy(pts, ptp)
            nc.tensor.matmul(cps, pts, eg_sb[j],
                             start=(j == 0), stop=(j == len(s_chunks) - 1))
        cs = p3.tile([128, SS], FP)
        nc.vector.tensor_scalar_mul(cs, cps, rinv)
        nc.sync.dma_start(out=ctx_out, in_=cs[:T_C, :])

    nc.compile()
    return nc


_CACHE = {}


def kernel(hidden_states, encoder_outputs, encoder_masks, W, b, v):
    hidden_states = np.asarray(hidden_states, dtype=np.float32)
    encoder_outputs = np.asarray(encoder_outputs, dtype=np.float32)
    encoder_masks = np.asarray(encoder_masks)
    W = np.asarray(W, dtype=np.float32)
    b = np.asarray(b, dtype=np.float32)
    v = np.asarray(v, dtype=np.float32)

    kept = [np.flatnonzero(encoder_masks[bb] != 0) for bb in range(B)]
    # Pad to a multiple of 4 (keeps all slices 4B-aligned for fp16 tiles);
    # every extra column costs ~0.7us across the 3 engines.
    S_c = max(8, -(-max(len(k) for k in kept) // 4) * 4)

    if S_c not in _CACHE:
        _CACHE[S_c] = _build(S_c)
    nc = _CACHE[S_c]

    weT = np.ascontiguousarray(W[:, TS:].T, dtype=np.float16)
    whT = np.ascontiguousarray(W[:, :TS].T, dtype=np.float16)
    v4 = np.ascontiguousarray(v.reshape(O_TILES, 128).T, dtype=np.float16)
    b4 = np.ascontiguousarray(b.reshape(O_TILES, 128).T)
    ident = np.eye(128, dtype=np.float32)

    in_maps = []
    for c in range(N_CORES):
        bb, th = c // 2, c % 2
        idx = kept[bb]
        k = len(idx)
        eg = np.zeros((S_c, SS), np.float32)
        eg[:k] = encoder_outputs[bb][idx]
        mrow = np.zeros((S_c,), np.float32)
        mrow[:k] = 1.0
        smalls = np.concatenate(
            [b4, np.broadcast_to(mrow, (128, S_c)), ident], axis=1)
        in_maps.append({
            "weT": weT,
            "whT": whT,
            "eTg": np.ascontiguousarray(eg.T, dtype=np.float16),
            "eg": eg,
            "hT": np.ascontiguousarray(
                hidden_states[bb, th * T_C:(th + 1) * T_C, :].T,
                dtype=np.float16),
            "v4": v4,
            "smalls": np.ascontiguousarray(smalls),
        })

    res = None
    for attempt in range(3):
        try:
            res = bass_utils.run_bass_kernel_spmd(nc, in_maps,
                                                  core_ids=list(range(N_CORES)))
            break
        except Exception:
            if attempt == 2:
                raise

    context = np.zeros((B, TGT, SS), np.float32)
    probs = np.zeros((B, TGT, SRC), np.float32)
    for c in range(N_CORES):
        bb, th = c // 2, c % 2
        idx = kept[bb]
        k = len(idx)
        r = res.results[c]
        context[bb, th * T_C:(th + 1) * T_C, :] = r["ctx_out"]
        probs[bb, th * T_C:(th + 1) * T_C, idx] = r["probs_out"][:, :k].T
    return context, probs


# revision 28
# speedup vs baseline: 1.0054x; 1.0054x over previous
"""Luong 'concat' attention (nn_LuongAttention) — Trainium2 Bass kernel.

Strategy (8 NeuronCores, no collectives):
  - Shard (batch, tgt): core c handles batch c//2, tgt rows (c%2)*128..+128.
  - Sparse-attention mask compaction on host: only src positions with
    mask==1 (plus padding to a multiple of 16) are shipped/computed.
    Padding columns are zeroed via an explicit mask multiply before the
    softmax normalization, so results are exact for any mask.
  - Per core the device computes:
      e_projT[o,s] = We @ e_g.T           (PE, fp16 in / fp32 acc)
      hbT[o,t]    = Wh @ h.T + b          (PE + DVE)
      energy[o, s*T+t] = hbT[:,t] + e_projT[:,s]   (DVE tensor_scalar, fp16)
      tanh(energy)                         (ACT, big slabs, fp16 out)
      score[t,s] = sum_o v[o]*tanh(...)    (PE: fp16 weights, rhs=v, fp32 acc)
      masked softmax over s                (DVE/ACT, fp32)
      context = probs @ e_g                (PE, fp32)
  - Host scatters compacted probs back to [B,TGT,SRC] and stacks context.

fp16 is used only for the energy mid-section: fp32 matmuls on TRN2 run as
two LOW/HIGH passes with per-pass LDWEIGHTS, which makes the per-(s,o_tile)
energy-as-weights reduction ~6x more expensive than fp16 (which gets FWL
and a single pass). Softmax + context stay fp32.
"""

import numpy as np
from contextlib import ExitStack

import concourse.bass as bass
import concourse.tile as tile
from concourse import bacc, mybir
from concourse import bass_utils

N_CORES = 8
B, TGT, SRC, SS, TS = 4, 256, 256, 512, 512
T_C = TGT * B // N_CORES  # tgt rows per core = 128
O_TILES = TS // 128  # 4
D_TILES = SS // 128  # 4
SLAB_S = 32   # src positions per (slab, o_tile) DVE/ACT chunk
FUSED_S = 18  # s-columns whose add+tanh run fused on ACT (DVE/ACT balance)

FP = mybir.dt.float32
HP = mybir.dt.float16


def _chunks(total, size):
    out = []
    off = 0
    while off < total:
        out.append((off, min(size, total - off)))
        off += size
    return out


def _build(S_c):
    """Trace + compile the per-core program for compacted src size S_c."""
    nc = bacc.Bacc("TRN2", target_bir_lowering=False, debug=False,
                   num_devices=N_CORES)

    weT = nc.dram_tensor("weT", [SS, TS], HP, kind="ExternalInput").ap()
    whT = nc.dram_tensor("whT", [TS, TS], HP, kind="ExternalInput").ap()
    eTg = nc.dram_tensor("eTg", [SS, S_c], HP, kind="ExternalInput").ap()
    eg = nc.dram_tensor("eg", [S_c, SS], FP, kind="ExternalInput").ap()
    hT = nc.dram_tensor("hT", [TS, T_C], HP, kind="ExternalInput").ap()
    v4 = nc.dram_tensor("v4", [128, O_TILES], HP, kind="ExternalInput").ap()
    # smalls = [b4 | maskrep | identity] merged into one fp32 DMA
    smalls = nc.dram_tensor("smalls", [128, O_TILES + S_c + 128], FP,
                            kind="ExternalInput").ap()
    ctx_out = nc.dram_tensor("ctx_out", [T_C, SS], FP, kind="ExternalOutput").ap()
    probs_out = nc.dram_tensor("probs_out", [T_C, S_c], FP,
                               kind="ExternalOutput").ap()

    s_chunks = _chunks(S_c, 128)

    with tile.TileContext(nc) as tc, ExitStack() as ctx:
        consts = ctx.enter_context(tc.tile_pool(name="consts", bufs=1))
        slabs = ctx.enter_context(tc.tile_pool(name="slabs", bufs=4))
        p3 = ctx.enter_context(tc.tile_pool(name="p3", bufs=1))
        psum = ctx.enter_context(tc.tile_pool(name="psum", bufs=1, space="PSUM"))
        psum2 = ctx.enter_context(tc.tile_pool(name="psum2", bufs=2, space="PSUM"))

        # ---- Phase 0: loads ------------------------------------------------
        # Big loads on different engines' DMA queues so they run in parallel.
        weT_sb = consts.tile([128, D_TILES, TS], HP)
        nc.sync.dma_start(out=weT_sb, in_=weT.rearrange("(dt p) o -> p dt o", p=128))
        whT_sb = consts.tile([128, D_TILES, TS], HP)
        nc.gpsimd.dma_start(out=whT_sb, in_=whT.rearrange("(dt p) o -> p dt o", p=128))
        eTg_sb = consts.tile([128, D_TILES, S_c], HP)
        nc.scalar.dma_start(out=eTg_sb, in_=eTg.rearrange("(dt p) s -> p dt s", p=128))
        hT_sb = consts.tile([128, D_TILES, T_C], HP)
        nc.gpsimd.dma_start(out=hT_sb, in_=hT.rearrange("(dt p) t -> p dt t", p=128))
        v4_sb = consts.tile([128, O_TILES], HP)
        nc.sync.dma_start(out=v4_sb, in_=v4)
        smalls_sb = consts.tile([128, O_TILES + S_c + 128], FP)
        nc.gpsimd.dma_start(out=smalls_sb, in_=smalls)
        b4_sb = smalls_sb[:, 0:O_TILES]
        mask_sb = smalls_sb[:, O_TILES:O_TILES + S_c]
        ident = smalls_sb[:, O_TILES + S_c:]
        eg_sb = []
        for off, sz in s_chunks:
            t = consts.tile([sz, SS], FP, tag=f"eg{off}")
            nc.scalar.dma_start(out=t, in_=eg[off:off + sz, :])
            eg_sb.append(t)

        # ---- Phase 1: projections -----------------------------------------
        e_projT = []  # [128, S_c] fp16 per o_tile ; o = ot*128 + p
        hbT = []      # [128, T_C] fp16 per o_tile (h_proj + b)
        for ot in range(O_TILES):
            # h-projection first: its DVE b-add overlaps the e-proj matmuls,
            # so the first slab add can start sooner.
            ph = psum2.tile([128, T_C], FP, tag="ph_proj")
            for dt in range(D_TILES):
                nc.tensor.matmul(ph, whT_sb[:, dt, ot * 128:(ot + 1) * 128],
                                 hT_sb[:, dt, :],
                                 start=(dt == 0), stop=(dt == D_TILES - 1))
            ht = consts.tile([128, T_C], HP, tag=f"hbT{ot}")
            nc.vector.tensor_scalar_add(ht, ph, b4_sb[:, ot:ot + 1])
            hbT.append(ht)

            pe = psum2.tile([128, S_c], FP, tag="pe_proj")
            for dt in range(D_TILES):
                nc.tensor.matmul(pe, weT_sb[:, dt, ot * 128:(ot + 1) * 128],
                                 eTg_sb[:, dt, :],
                                 start=(dt == 0), stop=(dt == D_TILES - 1))
            # fp32: tensor_scalar requires a float32 scalar operand
            et = consts.tile([128, S_c], FP, tag=f"eprojT{ot}")
            nc.scalar.copy(out=et, in_=pe)
            e_projT.append(et)

        # ---- Phase 2: energy / tanh / score -------------------------------
        # s-columns [0, n_slab) take the DVE-add + big-slab-ACT-tanh path;
        # s-columns [n_slab, S_c) run add+tanh fused in one ACT op each
        # (bias = e_projT column), balancing DVE vs ACT occupancy.
        n_fused = min(FUSED_S, S_c // 2)
        n_slab = S_c - n_fused
        score = psum.tile([128, S_c], FP)  # [t, s]
        mm_count = [0]

        def score_mm(s, ot, lhsT):
            # One accumulation group for the whole phase: start=True clears
            # the entire PSUM bank, so only the very first matmul carries it.
            i = mm_count[0]
            nc.tensor.matmul(score[:, s:s + 1], lhsT, v4_sb[:, ot:ot + 1],
                             start=(i == 0), stop=(i == S_c * O_TILES - 1))
            mm_count[0] = i + 1

        def fused_unit(s):
            for ot in range(O_TILES):
                ef = slabs.tile([128, T_C], HP, tag="efused")
                nc.scalar.activation(out=ef, in_=hbT[ot],
                                     func=mybir.ActivationFunctionType.Tanh,
                                     bias=e_projT[ot][:, s:s + 1], scale=1.0)
                score_mm(s, ot, ef)

        # Interleave: fused s-columns are spread across slab iterations so
        # ACT's fused work overlaps DVE's slab adds instead of tailing.
        slab_starts = list(range(0, n_slab, SLAB_S))
        fused_cols = list(range(n_slab, S_c))
        for k, s0 in enumerate(slab_starts):
            ns = min(SLAB_S, n_slab - s0)
            for ot in range(O_TILES):
                ein = slabs.tile([128, SLAB_S * T_C], HP, tag="ein")
                for j in range(ns):
                    nc.vector.tensor_scalar_add(
                        ein[:, j * T_C:(j + 1) * T_C], hbT[ot],
                        e_projT[ot][:, s0 + j:s0 + j + 1])
                eout = slabs.tile([128, SLAB_S * T_C], HP, tag="eout")
                nc.scalar.activation(out=eout[:, :ns * T_C], in_=ein[:, :ns * T_C],
                                     func=mybir.ActivationFunctionType.Tanh)
                for j in range(ns):
                    score_mm(s0 + j, ot, eout[:, j * T_C:(j + 1) * T_C])
                # Front-load fused columns into the earlier slots so ACT's
                # independent work fills pipeline-fill gaps instead of
                # trailing after DVE finishes.
                idx = k * O_TILES + ot
                eff = max(1, len(slab_starts) * O_TILES - 4)
                if idx < eff:
                    lo = len(fused_cols) * idx // eff
                    hi = len(fused_cols) * (idx + 1) // eff
                    for s in fused_cols[lo:hi]:
                        fused_unit(s)

        # ---- Phase 3: masked softmax + context ----------------------------
        m = p3.tile([128, 1], FP)
        nc.vector.reduce_max(out=m, in_=score, axis=mybir.AxisListType.X)
        negm = p3.tile([128, 1], FP)
        nc.vector.tensor_scalar_mul(negm, m, -1.0)
        pm = p3.tile([128, S_c], FP)  # exp(score - max), then masked
        nc.scalar.activation(out=pm, in_=score,
                             func=mybir.ActivationFunctionType.Exp,
                             bias=negm, scale=1.0)
        nc.vector.tensor_mul(pm, pm, mask_sb)
        ssum = p3.tile([128, 1], FP)
        nc.vector.reduce_sum(out=ssum, in_=pm, axis=mybir.AxisListType.X)
        # Guard a fully-masked row (sum==0 -> inf -> NaN); reference yields 0.
        nc.vector.tensor_scalar_max(ssum, ssum, 1e-30)
        rinv = p3.tile([128, 1], FP)
        nc.vector.reciprocal(rinv, ssum)

        pr = p3.tile([128, S_c], FP)
        nc.vector.tensor_scalar_mul(pr, pm, rinv)
        nc.sync.dma_start(out=probs_out, in_=pr[:T_C, :])

        cps = psum.tile([128, SS], FP)
        for j, (off, sz) in enumerate(s_chunks):
            ptp = psum2.tile([sz, 128], FP, tag="ptp")
            nc.tensor.transpose(ptp, pm[:, off:off + sz], ident)
            pts = p3.tile([sz, 128], FP, tag=f"pts{j}")
            nc.vector.tensor_copy(pts, ptp)
            nc.tensor.matmul(cps, pts, eg_sb[j],
                             start=(j == 0), stop=(j == len(s_chunks) - 1))
        cs = p3.tile([128, SS], FP)
        nc.vector.tensor_scalar_mul(cs, cps, rinv)
        nc.sync.dma_start(out=ctx_out, in_=cs[:T_C, :])

    nc.compile()
    return nc


_CACHE = {}


def kernel(hidden_states, encoder_outputs, encoder_masks, W, b, v):
    hidden_states = np.asarray(hidden_states, dtype=np.float32)
    encoder_outputs = np.asarray(encoder_outputs, dtype=np.float32)
    encoder_masks = np.asarray(encoder_masks)
    W = np.asarray(W, dtype=np.float32)
    b = np.asarray(b, dtype=np.float32)
    v = np.asarray(v, dtype=np.float32)

    kept = [np.flatnonzero(encoder_masks[bb] != 0) for bb in range(B)]
    # Pad to a multiple of 4 (keeps all slices 4B-aligned for fp16 tiles);
    # every extra column costs ~0.7us across the 3 engines.
    S_c = max(8, -(-max(len(k) for k in kept) // 4) * 4)

    if S_c not in _CACHE:
        _CACHE[S_c] = _build(S_c)
    nc = _CACHE[S_c]

    weT = np.ascontiguousarray(W[:, TS:].T, dtype=np.float16)
    whT = np.ascontiguousarray(W[:, :TS].T, dtype=np.float16)
    v4 = np.ascontiguousarray(v.reshape(O_TILES, 128).T, dtype=np.float16)
    b4 = np.ascontiguousarray(b.reshape(O_TILES, 128).T)
    ident = np.eye(128, dtype=np.float32)

    in_maps = []
    for c in range(N_CORES):
        bb, th = c // 2, c % 2
        idx = kept[bb]
        k = len(idx)
        eg = np.zeros((S_c, SS), np.float32)
        eg[:k] = encoder_outputs[bb][idx]
        mrow = np.zeros((S_c,), np.float32)
        mrow[:k] = 1.0
        smalls = np.concatenate(
            [b4, np.broadcast_to(mrow, (128, S_c)), ident], axis=1)
        in_maps.append({
            "weT": weT,
            "whT": whT,
            "eTg": np.ascontiguousarray(eg.T, dtype=np.float16),
            "eg": eg,
            "hT": np.ascontiguousarray(
                hidden_states[bb, th * T_C:(th + 1) * T_C, :].T,
                dtype=np.float16),
            "v4": v4,
            "smalls": np.ascontiguousarray(smalls),
        })

    res = None
    for attempt in range(3):
        try:
            res = bass_utils.run_bass_kernel_spmd(nc, in_maps,
                                                  core_ids=list(range(N_CORES)))
            break
        except Exception:
            if attempt == 2:
                raise

    context = np.zeros((B, TGT, SS), np.float32)
    probs = np.zeros((B, TGT, SRC), np.float32)
    for c in range(N_CORES):
        bb, th = c // 2, c % 2
        idx = kept[bb]
        k = len(idx)
        r = res.results[c]
        context[bb, th * T_C:(th + 1) * T_C, :] = r["ctx_out"]
        probs[bb, th * T_C:(th + 1) * T_C, idx] = r["probs_out"][:, :k].T
    return context, probs
